# revision 3
# baseline (speedup 1.0000x reference)
"""GIN 2-layer message-passing network on 8 Trainium2 NeuronCores.

Strategy (matches the dst-partitioned sharding hint):
  - Nodes are split into 8 contiguous chunks of N/8; core c owns chunk c and
    all edges whose destination lands in it (plus one self-edge per node,
    which realizes the `+ 1*h_i` part of the GIN aggregate exactly).
  - segment_sum becomes, per core: dma_gather of source-node feature rows
    (bf16) in edge order, then a segment-sum on the tensor engine using
    one-hot matrices built on the vector engine (is_equal against an iota
    row), accumulating in PSUM per 128-destination-node tile:
        aggT[feat, dst] += G[edges, feat].T-contract-. onehot[edges, dst]
  - The MLP runs in "transposed land" ([feat, nodes] layout) so activations
    never need transposing between matmuls; per tile:
        h = relu(w.T @ aggT + b)  via PE matmul + ACT relu-with-bias.
  - Between the two GIN layers the per-core h chunks are exchanged with 4
    AllGathers (one per quarter of each core's rows) so layer-2 gathers can
    index any node with int16 indices (< 32768 rows per gather source).
  - log_softmax of the final [40, nodes] tile is done after a PE transpose
    back to [nodes, 40]: row-max, subtract, exp-with-accumulated-sum (ACT),
    ln, subtract.

All per-core variability lives in the *data* (index / one-hot-column arrays,
padded to a per-group max across cores) so a single SPMD NEFF serves all 8
cores.
"""

import os
import sys

sys.path.insert(0, "/opt/trn_rl_repo")
sys.path.insert(0, "/opt/trn_rl_repo/concourse")
os.environ.setdefault("TRN_TYPE", "TRN2")

import numpy as np
import ml_dtypes

BF16 = ml_dtypes.bfloat16

NCORES = 8


class Cfg:
    def __init__(self, n, feat, hid, cls, tiles_per_batch=5):
        assert n % (NCORES * 4) == 0
        self.N = n
        self.F = feat          # input feature dim (must be 128 here)
        self.H = hid           # hidden dim (128)
        self.CLS = cls         # classes
        self.NPC = n // NCORES          # nodes per core
        self.QROWS = self.NPC // 4      # rows per quarter per core
        self.SRCROWS = self.QROWS * NCORES  # rows per gather source tensor
        self.NT = -(-self.NPC // 128)   # dst tiles per core
        self.last_rows = self.NPC - (self.NT - 1) * 128
        self.B = tiles_per_batch


FULL = Cfg(100000, 128, 128, 40, tiles_per_batch=5)


def _prep_graph(edge_index, cfg):
    """Host-side sharding: returns (schedule, per-core index arrays).

    schedule: dict with
      C[t][q]        chunk count per (tile, quarter) group (max over cores)
      batches        list of lists of tile ids
      call_slots[b][q], call_chunk_off maps for emission
    per-core: gidx_wr [128, TOT//16] int16, dstloc [128, TOT//128] bf16
    """
    N, NPC, QROWS, NT = cfg.N, cfg.NPC, cfg.QROWS, cfg.NT
    src = np.asarray(edge_index[0], dtype=np.int64)
    dst = np.asarray(edge_index[1], dtype=np.int64)
    # self-edges give the +h_i term of the GIN aggregate
    allid = np.arange(N, dtype=np.int64)
    src = np.concatenate([src, allid])
    dst = np.concatenate([dst, allid])

    core = dst // NPC
    per_core = []
    counts = np.zeros((NCORES, NT * 4), np.int64)
    for c in range(NCORES):
        m = core == c
        s = src[m]
        dloc = (dst[m] - c * NPC).astype(np.int64)
        t = dloc >> 7
        q = (s % NPC) // QROWS
        gidxv = (s // NPC) * QROWS + (s % QROWS)
        dstin = dloc & 127
        gid = (t * 4 + q).astype(np.int64)
        counts[c] = np.bincount(gid, minlength=NT * 4)
        per_core.append((gid, gidxv.astype(np.int32), dstin.astype(np.int32)))

    cmax = counts.max(axis=0)                       # [NT*4]
    C = -(-cmax // 128)                             # chunks per (t,q)
    slots = C * 128                                 # slots per (t,q)
    # batches of tiles
    B = cfg.B
    batches = [list(range(b, min(b + B, NT))) for b in range(0, NT, B)]
    # slot offsets in (b, q, t) order
    off = 0
    slot_off = np.zeros(NT * 4, np.int64)
    call_slots = []           # [b][q] -> num slots in that gather call
    call_off = []             # [b][q] -> slot offset of call start
    for tiles in batches:
        cs, co = [], []
        for q in range(4):
            co.append(off)
            s0 = off
            for t in tiles:
                slot_off[t * 4 + q] = off
                off += slots[t * 4 + q]
            cs.append(off - s0)
        call_slots.append(cs)
        call_off.append(co)
    tot = off
    assert tot % 128 == 0

    gidx_all, dstloc_all = [], []
    for c in range(NCORES):
        gid, gidxv, dstin = per_core[c]
        order = np.argsort(gid, kind="stable")
        gs = gid[order]
        cnt = counts[c]
        starts = np.zeros(NT * 4, np.int64)
        np.cumsum(cnt[:-1], out=starts[1:])
        rank = np.arange(len(gs)) - starts[gs]
        slot = slot_off[gs] + rank
        gflat = np.zeros(tot, np.int16)
        dflat = np.full(tot, 200.0, np.float32)
        gflat[slot] = gidxv[order].astype(np.int16)
        dflat[slot] = dstin[order]
        # wrap for dma_gather: [p, col] = gflat[col*16 + p%16], replicated x8
        gwr = np.tile(gflat.reshape(tot // 16, 16).T, (8, 1)).copy()
        dloc = dflat.reshape(tot // 128, 128).T.astype(BF16).copy()
        gidx_all.append(gwr)
        dstloc_all.append(dloc)

    sched = dict(C=C, slots=slots, batches=batches, call_slots=call_slots,
                 call_off=call_off, slot_off=slot_off, tot=tot)
    return sched, gidx_all, dstloc_all


def _perm_rows(x, cfg):
    """x [N, F] -> 4 arrays [SRCROWS, F]; source s holds global row
    g = r*NPC + s*QROWS + u at position r*QROWS + u."""
    N, NPC, QROWS = cfg.N, cfg.NPC, cfg.QROWS
    g = np.arange(N)
    s = (g % NPC) // QROWS
    pos = (g // NPC) * QROWS + (g % QROWS)
    out = []
    for si in range(4):
        m = s == si
        a = np.empty((cfg.SRCROWS, x.shape[1]), x.dtype)
        a[pos[m]] = x[m]
        out.append(a)
    return out


def _build_nc(cfg, sched, eps1, eps2):
    from concourse import mybir
    import concourse.bacc as bacc
    import concourse.tile as tile

    F, H, CLS, NT, NPC = cfg.F, cfg.H, cfg.CLS, cfg.NT, cfg.NPC
    C = sched["C"]
    batches = sched["batches"]
    call_slots = sched["call_slots"]
    tot = sched["tot"]
    f32 = mybir.dt.float32
    bf16 = mybir.dt.bfloat16
    AT = mybir.ActivationFunctionType
    OP = mybir.AluOpType

    assert eps1 == 0.0 and eps2 == 0.0, "nonzero eps not implemented"

    nc = bacc.Bacc("TRN2", target_bir_lowering=False, debug=False,
                   num_devices=NCORES)

    xq = [nc.dram_tensor(f"xq{q}", [cfg.SRCROWS, F], bf16, kind="ExternalInput")
          for q in range(4)]
    w1_t = nc.dram_tensor("w1", [F, H], f32, kind="ExternalInput")
    w2_t = nc.dram_tensor("w2", [H, H], f32, kind="ExternalInput")
    w3_t = nc.dram_tensor("w3", [H, H], f32, kind="ExternalInput")
    w4_t = nc.dram_tensor("w4", [H, CLS], f32, kind="ExternalInput")
    b1_t = nc.dram_tensor("b1", [H, 1], f32, kind="ExternalInput")
    b2_t = nc.dram_tensor("b2", [H, 1], f32, kind="ExternalInput")
    b3_t = nc.dram_tensor("b3", [H, 1], f32, kind="ExternalInput")
    b4_t = nc.dram_tensor("b4", [CLS, 1], f32, kind="ExternalInput")
    iota_t = nc.dram_tensor("iota", [128, 128], bf16, kind="ExternalInput")
    ident_t = nc.dram_tensor("ident", [128, 128], f32, kind="ExternalInput")
    gidx_t = nc.dram_tensor("gidx", [128, tot // 16], mybir.dt.int16,
                            kind="ExternalInput")
    dstloc_t = nc.dram_tensor("dstloc", [128, tot // 128], bf16,
                              kind="ExternalInput")
    out_t = nc.dram_tensor("out", [NPC, CLS], f32, kind="ExternalOutput")

    maxS = max(max(cs) for cs in call_slots)

    with tile.TileContext(nc) as tc:
        with tc.tile_pool(name="const", bufs=1) as cp, \
             tc.tile_pool(name="gp", bufs=6) as gp, \
             tc.tile_pool(name="ohp", bufs=6) as ohp, \
             tc.tile_pool(name="idxp", bufs=6) as idxp, \
             tc.tile_pool(name="dlp", bufs=6) as dlp, \
             tc.tile_pool(name="work", bufs=4) as wp, \
             tc.tile_pool(name="small", bufs=6) as sp, \
             tc.tile_pool(name="aggps", bufs=3, space="PSUM") as aggps, \
             tc.tile_pool(name="mmps", bufs=3, space="PSUM") as mmps, \
             tc.tile_pool(name="dram", bufs=1, space="DRAM") as dp:

            w1 = cp.tile([F, H], f32); nc.sync.dma_start(w1[:], w1_t.ap())
            w2 = cp.tile([H, H], f32); nc.sync.dma_start(w2[:], w2_t.ap())
            w3 = cp.tile([H, H], f32); nc.sync.dma_start(w3[:], w3_t.ap())
            w4 = cp.tile([H, CLS], f32); nc.sync.dma_start(w4[:], w4_t.ap())
            b1 = cp.tile([H, 1], f32); nc.sync.dma_start(b1[:], b1_t.ap())
            b2 = cp.tile([H, 1], f32); nc.sync.dma_start(b2[:], b2_t.ap())
            b3 = cp.tile([H, 1], f32); nc.sync.dma_start(b3[:], b3_t.ap())
            b4 = cp.tile([CLS, 1], f32); nc.sync.dma_start(b4[:], b4_t.ap())
            iota = cp.tile([128, 128], bf16); nc.sync.dma_start(iota[:], iota_t.ap())
            ident = cp.tile([128, 128], f32); nc.sync.dma_start(ident[:], ident_t.ap())

            h_own = dp.tile([NPC, H], bf16)
            ag_space = "Local" if os.environ.get("GIN_AG_LOCAL") else "Shared"
            no_ag = bool(os.environ.get("GIN_NO_AG"))
            h_ag = [tc.tile([cfg.SRCROWS, H], bf16, space="DRAM",
                            addr_space=ag_space, name=f"h_ag{s}")[0]
                    for s in range(4)]

            def layer(sources, is_first, batch_limit=None, skip_mlp=False):
                """Emit one GIN layer. sources: list of 4 gather-source APs."""
                ag_emitted = [False] * 4
                for b, tiles in enumerate(batches):
                    if batch_limit is not None and b >= batch_limit:
                        break
                    G, OH = [], []
                    for q in range(4):
                        S = call_slots[b][q]
                        if S == 0:
                            G.append(None); OH.append(None)
                            continue
                        o = sched["call_off"][b][q]
                        gi = idxp.tile([128, maxS // 16], mybir.dt.int16,
                                       tag="idx")
                        nc.sync.dma_start(gi[:, : S // 16],
                                          gidx_t.ap()[:, o // 16:(o + S) // 16])
                        dl = dlp.tile([128, maxS // 128], bf16, tag="dl")
                        nc.sync.dma_start(dl[:, : S // 128],
                                          dstloc_t.ap()[:, o // 128:(o + S) // 128])
                        g = gp.tile([128, maxS // 128, 128], bf16, tag="g")
                        nc.gpsimd.dma_gather(
                            g[:, : S // 128, :], sources[q], gi[:, : S // 16],
                            S, S, F, single_packet=False)
                        oh = ohp.tile([128, maxS // 128, 128], bf16, tag="oh")
                        nc.vector.tensor_tensor(
                            out=oh[:, : S // 128, :],
                            in0=iota[:].unsqueeze(1).broadcast_to(
                                [128, S // 128, 128]),
                            in1=dl[:, : S // 128].unsqueeze(2).broadcast_to(
                                [128, S // 128, 128]),
                            op=OP.is_equal)
                        G.append(g); OH.append(oh)

                    # per-(b,q) chunk offsets of each tile's group
                    chunk_off = [co // 128 for co in sched["call_off"][b]]
                    pos = [0, 0, 0, 0]
                    for t in tiles:
                        nch = [int(C[t * 4 + q]) for q in range(4)]
                        tot_ch = sum(nch)
                        if tot_ch == 0:
                            continue
                        agg = aggps.tile([128, 128], f32, tag="agg")
                        k = 0
                        for q in range(4):
                            base = pos[q]
                            for j in range(nch[q]):
                                col = base + j
                                nc.tensor.matmul(
                                    out=agg[:],
                                    lhsT=G[q][:, col, :],
                                    rhs=OH[q][:, col, :],
                                    start=(k == 0),
                                    stop=(k == tot_ch - 1))
                                k += 1
                            pos[q] += nch[q]
                        rows = 128 if t < NT - 1 else cfg.last_rows

                        aggT = wp.tile([128, 128], f32, tag="aggT")
                        nc.scalar.activation(out=aggT[:], in_=agg[:], func=AT.Copy)

                        if skip_mlp:
                            continue
                        if is_first:
                            ps1 = mmps.tile([128, 128], f32, tag="mm")
                            nc.tensor.matmul(out=ps1[:], lhsT=w1[:], rhs=aggT[:],
                                             start=True, stop=True)
                            h1 = wp.tile([128, 128], f32, tag="h1")
                            nc.scalar.activation(out=h1[:], in_=ps1[:],
                                                 func=AT.Relu, bias=b1[:])
                            ps2 = mmps.tile([128, 128], f32, tag="mm")
                            nc.tensor.matmul(out=ps2[:], lhsT=w2[:], rhs=h1[:],
                                             start=True, stop=True)
                            h2 = wp.tile([128, 128], f32, tag="h2")
                            nc.scalar.activation(out=h2[:], in_=ps2[:],
                                                 func=AT.Relu, bias=b2[:])
                            # transpose back to [nodes, feat], cast bf16, store
                            pst = mmps.tile([128, 128], f32, tag="mm")
                            nc.tensor.transpose(out=pst[:], in_=h2[:],
                                                identity=ident[:])
                            hbf = sp.tile([128, 128], bf16, tag="hbf")
                            nc.vector.tensor_copy(out=hbf[:], in_=pst[:])
                            nc.sync.dma_start(
                                h_own[:][t * 128: t * 128 + rows, :],
                                hbf[:rows, :])
                            # fire AllGathers as soon as their rows are done
                            for s in range(4):
                                if not ag_emitted[s] and \
                                        (t + 1) * 128 >= (s + 1) * cfg.QROWS:
                                    ag_emitted[s] = True
                                    if no_ag:
                                        continue
                                    nc.gpsimd.collective_compute(
                                        "AllGather", OP.bypass,
                                        replica_groups=[list(range(NCORES))],
                                        ins=[h_own[:][s * cfg.QROWS:
                                                      (s + 1) * cfg.QROWS, :]],
                                        outs=[h_ag[s][:]])
                        else:
                            ps1 = mmps.tile([128, 128], f32, tag="mm")
                            nc.tensor.matmul(out=ps1[:], lhsT=w3[:], rhs=aggT[:],
                                             start=True, stop=True)
                            h3 = wp.tile([128, 128], f32, tag="h1")
                            nc.scalar.activation(out=h3[:], in_=ps1[:],
                                                 func=AT.Relu, bias=b3[:])
                            ps2 = mmps.tile([128, 128], f32, tag="mm")
                            nc.tensor.matmul(out=ps2[:CLS, :128],
                                             lhsT=w4[:], rhs=h3[:],
                                             start=True, stop=True)
                            c4 = sp.tile([CLS, 128], f32, tag="c4")
                            nc.vector.tensor_tensor(
                                out=c4[:], in0=ps2[:CLS, :128],
                                in1=b4[:].broadcast_to([CLS, 128]),
                                op=OP.add)
                            psf = mmps.tile([128, 128], f32, tag="mm")
                            nc.tensor.transpose(out=psf[:128, :CLS], in_=c4[:],
                                                identity=ident[:CLS, :CLS])
                            mx = sp.tile([128, 1], f32, tag="mx")
                            nc.vector.tensor_reduce(
                                out=mx[:], in_=psf[:128, :CLS],
                                axis=mybir.AxisListType.X, op=OP.max)
                            tsh = sp.tile([128, CLS], f32, tag="tsh")
                            nc.vector.tensor_tensor(
                                out=tsh[:], in0=psf[:128, :CLS],
                                in1=mx[:].broadcast_to([128, CLS]),
                                op=OP.subtract)
                            esum = sp.tile([128, 1], f32, tag="esum")
                            edum = sp.tile([128, CLS], f32, tag="edum")
                            nc.scalar.activation(out=edum[:], in_=tsh[:],
                                                 func=AT.Exp, accum_out=esum[:])
                            lse = sp.tile([128, 1], f32, tag="lse")
                            nc.scalar.activation(out=lse[:], in_=esum[:],
                                                 func=AT.Ln)
                            osb = sp.tile([128, CLS], f32, tag="osb")
                            nc.vector.tensor_tensor(
                                out=osb[:], in0=tsh[:],
                                in1=lse[:].broadcast_to([128, CLS]),
                                op=OP.subtract)
                            nc.sync.dma_start(
                                out_t.ap()[t * 128: t * 128 + rows, :],
                                osb[:rows, :])

            l1b = int(os.environ.get("GIN_L1_BATCHES", "0"))
            if l1b:
                layer([x.ap() for x in xq], is_first=True,
                      batch_limit=l1b, skip_mlp=True)
            else:
                layer([x.ap() for x in xq], is_first=True)
                if no_ag:
                    layer([x.ap() for x in xq], is_first=False)
                else:
                    layer([h[:] for h in h_ag], is_first=False)

    nc.compile()
    return nc


LAST_BENCH_NS = None


def _exec_spmd_timed(nc, in_maps, iters):
    """jit-once SPMD exec (mirrors bass2jax.run_bass_via_pjrt) + steady-state
    timing of repeated NEFF executions. Returns per-core result dicts."""
    import time

    import jax
    import numpy as _np
    from jax.experimental.shard_map import shard_map
    from jax.sharding import Mesh, NamedSharding, PartitionSpec

    from concourse import bass2jax, mybir
    from concourse.bass2jax import _bass_exec_p, install_neuronx_cc_hook, \
        partition_id_tensor

    install_neuronx_cc_hook()
    n_cores = len(in_maps)
    partition_name = (nc.partition_id_tensor.name
                      if nc.partition_id_tensor else None)
    in_names, out_names, out_avals, zero_outs = [], [], [], []
    for alloc in nc.m.functions[0].allocations:
        if not isinstance(alloc, mybir.MemoryLocationSet):
            continue
        name = alloc.memorylocations[0].name
        if alloc.kind == "ExternalInput":
            if name != partition_name:
                in_names.append(name)
        elif alloc.kind == "ExternalOutput":
            out_names.append(name)
            shape = tuple(alloc.tensor_shape)
            dtype = mybir.dt.np(alloc.dtype)
            out_avals.append(jax.core.ShapedArray(shape, dtype))
            zero_outs.append(_np.zeros(shape, dtype))
    n_params = len(in_names)
    all_in_names = in_names + out_names
    if partition_name is not None:
        all_in_names = all_in_names + [partition_name]

    def _body(*args):
        operands = list(args)
        if partition_name is not None:
            operands.append(partition_id_tensor())
        outs = _bass_exec_p.bind(
            *operands,
            out_avals=tuple(out_avals),
            in_names=tuple(all_in_names),
            out_names=tuple(out_names),
            lowering_input_output_aliases=(),
            sim_require_finite=True,
            sim_require_nnan=True,
            nc=nc,
        )
        return tuple(outs)

    devices = jax.devices()[:n_cores]
    mesh = Mesh(_np.asarray(devices), ("core",))
    spec = PartitionSpec("core")
    n_outs = len(out_names)
    fn = jax.jit(
        shard_map(_body, mesh=mesh, in_specs=(spec,) * (n_params + n_outs),
                  out_specs=(spec,) * n_outs, check_rep=False),
        keep_unused=True,
    )
    sh = NamedSharding(mesh, spec)
    concat_in = [
        jax.device_put(
            _np.concatenate([_np.asarray(in_maps[c][nm]) for c in
                             range(n_cores)], axis=0), sh)
        for nm in in_names
    ]
    concat_zeros = [
        jax.device_put(_np.zeros((n_cores * z.shape[0], *z.shape[1:]),
                                 z.dtype), sh)
        for z in zero_outs
    ]
    out = jax.block_until_ready(fn(*concat_in, *concat_zeros))
    times = []
    for _ in range(max(iters, 1)):
        t0 = time.perf_counter()
        r = jax.block_until_ready(fn(*concat_in, *concat_zeros))
        times.append((time.perf_counter() - t0) * 1e9)
    global LAST_BENCH_NS
    LAST_BENCH_NS = times
    res = []
    for c in range(n_cores):
        res.append({
            nm: _np.asarray(out[i]).reshape(n_cores, *out_avals[i].shape)[c]
            for i, nm in enumerate(out_names)
        })
    return res


def _run(inputs, cfg):
    from concourse.bass_utils import run_bass_kernel_spmd

    x = np.asarray(inputs["x"], np.float32)
    edge_index = np.asarray(inputs["edge_index"])
    eps1 = float(np.asarray(inputs["eps1"]))
    eps2 = float(np.asarray(inputs["eps2"]))

    sched, gidx_all, dstloc_all = _prep_graph(edge_index, cfg)
    xqs = _perm_rows(x.astype(BF16), cfg)

    nc = _build_nc(cfg, sched, eps1, eps2)

    iota_np = np.tile(np.arange(128, dtype=np.float32),
                      (128, 1)).astype(BF16)
    ident_np = np.eye(128, dtype=np.float32)
    base = {
        "w1": np.asarray(inputs["w1"], np.float32),
        "w2": np.asarray(inputs["w2"], np.float32),
        "w3": np.asarray(inputs["w3"], np.float32),
        "w4": np.asarray(inputs["w4"], np.float32),
        "b1": np.asarray(inputs["b1"], np.float32).reshape(-1, 1),
        "b2": np.asarray(inputs["b2"], np.float32).reshape(-1, 1),
        "b3": np.asarray(inputs["b3"], np.float32).reshape(-1, 1),
        "b4": np.asarray(inputs["b4"], np.float32).reshape(-1, 1),
        "iota": iota_np,
        "ident": ident_np,
    }
    for q in range(4):
        base[f"xq{q}"] = np.ascontiguousarray(xqs[q])

    in_maps = []
    for c in range(NCORES):
        m = dict(base)
        m["gidx"] = gidx_all[c]
        m["dstloc"] = dstloc_all[c]
        in_maps.append(m)

    bench = int(os.environ.get("GIN_BENCH", "0"))
    if bench:
        results = _exec_spmd_timed(nc, in_maps, bench)
    else:
        res = run_bass_kernel_spmd(nc, in_maps, core_ids=list(range(NCORES)))
        results = res.results
    out = np.concatenate([r["out"] for r in results], axis=0)
    return out.astype(np.float32)


def kernel(**inputs):
    return _run(inputs, FULL)

